# revision 1
# baseline (speedup 1.0000x reference)
"""Megatron-style tensor-parallel causal attention (BitLinear qkv/o) on 8 TRN2 cores.

Sharding: each core owns 2 of 16 heads (qkv_weight rows) and the matching
256 o_weight columns. x/rotary replicated; partial outputs summed on host.
Matmuls run as float32r (FP22 truncated fp32) at full PE rate with fp32 PSUM
accumulation. Quantized weights are small integers => exact in FP22.
"""

import math

import numpy as np

EPS = 1e-5
NUM_HEADS = 16
HEAD_DIM = 128
B, S, H = 2, 2048, 2048
NCORES = 8
HPC = NUM_HEADS // NCORES        # heads per core = 2
FPC = 3 * HPC * HEAD_DIM         # qkv features per core = 768
P = 128
NHT = H // P                     # 16 h_in tiles
CH = 256                         # proj token chunk
NCH = S // CH                    # 8 chunks per batch
QC = 512                         # attention q chunk
NQC = S // QC                    # 4


def _build_program():
    import concourse.bacc as bacc
    import concourse.mybir as mybir
    import concourse.tile as tile

    f32 = mybir.dt.float32
    f32r = mybir.dt.float32r
    AF = mybir.ActivationFunctionType

    nc = bacc.Bacc(None, target_bir_lowering=False)

    xt = nc.dram_tensor("xt", [B, H, S], f32, kind="ExternalInput")
    wqkv = nc.dram_tensor("wqkv", [H, FPC], f32, kind="ExternalInput")
    wo = nc.dram_tensor("wo", [HPC * HEAD_DIM, H], f32, kind="ExternalInput")
    cos_t = nc.dram_tensor("cos_t", [P, S], f32, kind="ExternalInput")
    sin_s = nc.dram_tensor("sin_s", [P, S], f32, kind="ExternalInput")
    masks = nc.dram_tensor("masks", [P, 4 * QC + P], f32, kind="ExternalInput")
    out = nc.dram_tensor("out", [B, S, H], f32, kind="ExternalOutput")

    def r(ap):
        return ap.bitcast(f32r)

    with tile.TileContext(nc) as tc:
        with tc.tile_pool(name="const", bufs=1) as cpool:
            w_sb = cpool.tile([P, NHT, FPC], f32r)
            nc.sync.dma_start(w_sb[:], wqkv.rearrange("(t p) f -> p t f", p=P).bitcast(f32r))
            wo_sb = cpool.tile([P, 2, H], f32r)
            nc.sync.dma_start(wo_sb[:], wo.rearrange("(t p) o -> p t o", p=P).bitcast(f32r))
            masks_sb = cpool.tile([P, 4 * QC + P], f32r)
            nc.sync.dma_start(masks_sb[:], masks[:].bitcast(f32r))

            rot_sb = cpool.tile([P, 2 * S], f32r)
            nc.sync.dma_start(rot_sb[:, 0:S], cos_t[:].bitcast(f32r))
            nc.sync.dma_start(rot_sb[:, S : 2 * S], sin_s[:].bitcast(f32r))

            for b in range(B):
                with tc.tile_pool(name=f"bat{b}", bufs=1) as bpool:
                    # qk[f]: roped q/k tiles [d, s]; f = (q0,q1,k0,k1)
                    qk = [bpool.tile([P, S], f32r, tag=f"qk{f}", name=f"qk{f}_{b}") for f in range(4)]
                    v_sb = bpool.tile([P, S * HPC], f32r, tag="v")

                    with (
                        tc.tile_pool(name=f"proj{b}", bufs=2) as ppool,
                        tc.psum_pool(name=f"pps{b}", bufs=4) as pps,
                    ):
                        for tcn in range(NCH):
                            xt_sb = ppool.tile([P, NHT, CH], f32r, tag="xt")
                            nc.sync.dma_start(
                                xt_sb[:],
                                xt[b, :, tcn * CH : (tcn + 1) * CH].rearrange(
                                    "(t p) c -> p t c", p=P
                                ).bitcast(f32r),
                            )
                            for f in range(4):
                                ps = pps.tile([P, CH], f32, tag="qk")
                                for h in range(NHT):
                                    nc.tensor.matmul(
                                        ps[:],
                                        lhsT=(w_sb[:, h, f * P : (f + 1) * P]),
                                        rhs=(xt_sb[:, h, :]),
                                        start=(h == 0),
                                        stop=(h == NHT - 1),
                                    )
                                nc.any.tensor_copy(
                                    qk[f][:, tcn * CH : (tcn + 1) * CH], ps[:]
                                )
                            for tsub in range(2):
                                psv = pps.tile([P, 2 * P], f32, tag="v")
                                for h in range(NHT):
                                    nc.tensor.matmul(
                                        psv[:],
                                        lhsT=(xt_sb[:, h, tsub * P : (tsub + 1) * P]),
                                        rhs=(w_sb[:, h, 4 * P : 6 * P]),
                                        start=(h == 0),
                                        stop=(h == NHT - 1),
                                    )
                                kb = 2 * tcn + tsub
                                nc.any.tensor_copy(
                                    v_sb[:, kb * 2 * P : (kb + 1) * 2 * P], psv[:]
                                )
                        # RoPE in place on q/k tiles
                        for f in range(4):
                            m1 = ppool.tile([P, S], f32r, tag="m1", bufs=1)
                            qsw = ppool.tile([P, S], f32r, tag="qsw", bufs=1)
                            tmp = ppool.tile([P, S], f32r, tag="tmp", bufs=1)
                            nc.sync.dma_start(qsw[0:64, :], qk[f][64:128, :])
                            nc.sync.dma_start(qsw[64:128, :], qk[f][0:64, :])
                            nc.vector.tensor_mul(m1[:], qk[f][:], rot_sb[:, 0:S])
                            nc.vector.tensor_mul(tmp[:], qsw[:], rot_sb[:, S : 2 * S])
                            nc.vector.tensor_add(qk[f][:], m1[:], tmp[:])

                    with (
                        tc.tile_pool(name=f"attn{b}", bufs=2) as apool,
                        tc.psum_pool(name=f"aps{b}", bufs=1) as aps,
                    ):
                        for qc in range(NQC):
                            kmax = 4 * qc + 4  # causal k-tile count
                            yn = []
                            for hl in range(2):
                                yt_ps = aps.tile([P, QC], f32, tag="yt")
                                sum_ps = aps.tile([P, QC], f32, tag="sum")
                                for g in range(0, kmax, 2):
                                    sc_ps = aps.tile([P, 2 * QC], f32, tag="sc", bufs=2)
                                    for j2 in range(2):
                                        kb = g + j2
                                        nc.tensor.matmul(
                                            sc_ps[:, j2 * QC : (j2 + 1) * QC],
                                            lhsT=(qk[2 + hl][:, kb * P : (kb + 1) * P]),
                                            rhs=(qk[hl][:, qc * QC : (qc + 1) * QC]),
                                            start=True,
                                            stop=True,
                                        )
                                    ex = apool.tile([P, 2 * QC], f32r, tag="ex", bufs=3)
                                    nc.scalar.activation(ex[:], sc_ps[:], AF.Exp)
                                    for j2 in range(2):
                                        kb = g + j2
                                        if kb >= 4 * qc:
                                            jj = kb - 4 * qc
                                            nc.gpsimd.tensor_mul(
                                                ex[:, j2 * QC : (j2 + 1) * QC],
                                                ex[:, j2 * QC : (j2 + 1) * QC],
                                                masks_sb[:, jj * QC : (jj + 1) * QC],
                                            )
                                    for j2 in range(2):
                                        kb = g + j2
                                        nc.tensor.matmul(
                                            yt_ps[:],
                                            lhsT=(v_sb[:, kb * 2 * P + hl * P : kb * 2 * P + (hl + 1) * P]),
                                            rhs=(ex[:, j2 * QC : (j2 + 1) * QC]),
                                            start=(kb == 0),
                                            stop=(kb == kmax - 1),
                                        )
                                        nc.tensor.matmul(
                                            sum_ps[:],
                                            lhsT=(masks_sb[:, 4 * QC : 4 * QC + P]),
                                            rhs=(ex[:, j2 * QC : (j2 + 1) * QC]),
                                            start=(kb == 0),
                                            stop=(kb == kmax - 1),
                                        )
                                recip = apool.tile([P, QC], f32, tag="rc")
                                nc.vector.reciprocal(recip[:], sum_ps[:])
                                y = apool.tile([P, QC], f32r, tag=f"yn{hl}")
                                nc.vector.tensor_mul(y[:], yt_ps[:], recip[:])
                                yn.append(y)
                            for tt in range(4):
                                for oc in range(4):
                                    ops = aps.tile([P, QC], f32, tag="op", bufs=2)
                                    for hl in range(2):
                                        nc.tensor.matmul(
                                            ops[:],
                                            lhsT=(yn[hl][:, tt * P : (tt + 1) * P]),
                                            rhs=(wo_sb[:, hl, oc * QC : (oc + 1) * QC]),
                                            start=(hl == 0),
                                            stop=(hl == 1),
                                        )
                                    os_sb = apool.tile([P, QC], f32, tag="os", bufs=4)
                                    if (tt + oc) % 2 == 0:
                                        nc.vector.tensor_copy(os_sb[:], ops[:])
                                    else:
                                        nc.scalar.copy(os_sb[:], ops[:])
                                    nc.sync.dma_start(
                                        out[
                                            b,
                                            qc * QC + tt * P : qc * QC + (tt + 1) * P,
                                            oc * QC : (oc + 1) * QC,
                                        ],
                                        os_sb[:],
                                    )
    nc.finalize()
    return nc


_NC_CACHE = None


def _get_program():
    global _NC_CACHE
    if _NC_CACHE is None:
        _NC_CACHE = _build_program()
    return _NC_CACHE


def kernel(x, rotary, qkv_weight, o_weight):
    import jax
    import jax.numpy as jnp
    from concourse.bass_utils import run_bass_kernel_spmd

    cpu = jax.devices("cpu")[0]
    with jax.default_device(cpu):
        sq = jnp.mean(jnp.abs(jnp.asarray(qkv_weight)))
        wq_q = np.asarray(jnp.round(jnp.asarray(qkv_weight) / (sq + EPS)), np.float32)
        so = jnp.mean(jnp.abs(jnp.asarray(o_weight)))
        wo_q = np.asarray(jnp.round(jnp.asarray(o_weight) / (so + EPS)), np.float32)
        sq = float(sq)
        so = float(so)

    xt = np.ascontiguousarray(x.transpose(0, 2, 1)).astype(np.float32)
    cos_t = np.ascontiguousarray(rotary[1].T).astype(np.float32)
    sin_t = np.ascontiguousarray(rotary[0].T).astype(np.float32)
    sin_s = sin_t.copy()
    sin_s[:64] *= -1.0

    mask = np.zeros((P, 4 * QC + P), np.float32)
    kk = np.arange(P)[:, None]
    qq = np.arange(QC)[None, :]
    for j in range(4):
        mask[:, j * QC : (j + 1) * QC] = (qq >= j * P + kk).astype(np.float32)
    mask[:, 4 * QC :] = 1.0

    sm_scale = np.float32(sq * sq / math.sqrt(HEAD_DIM))
    final_scale = sq * so

    in_maps = []
    for c in range(NCORES):
        # feature order per core: q_h0, q_h1, k_h0, k_h1, v_h0, v_h1 (128 each)
        # softmax scale is folded into the q rows (scores = (q*sm)·k).
        rows = []
        for part in range(3):  # q, k, v blocks of qkv_weight
            for hl in range(HPC):
                g = 2 * c + hl
                blk = wq_q[part * H + g * HEAD_DIM : part * H + (g + 1) * HEAD_DIM]
                if part == 0:
                    blk = blk * sm_scale
                rows.append(blk)
        wqkv_c = np.ascontiguousarray(np.concatenate(rows, axis=0).T).astype(np.float32)  # [H, 768]
        wo_c = np.ascontiguousarray(
            (wo_q[:, c * FPC // 3 : (c + 1) * FPC // 3].T * final_scale).astype(
                np.float32
            )
        )  # [256, H]
        in_maps.append(
            {
                "xt": xt,
                "wqkv": wqkv_c,
                "wo": wo_c,
                "cos_t": cos_t,
                "sin_s": sin_s,
                "masks": mask,
            }
        )

    nc = _get_program()
    res = run_bass_kernel_spmd(nc, in_maps, core_ids=list(range(NCORES)))
    acc = res.results[0]["out"].astype(np.float32)
    for c in range(1, NCORES):
        acc = acc + res.results[c]["out"]
    return acc

